# revision 4
# baseline (speedup 1.0000x reference)
"""Trainium2 Bass kernel for nn_DKEncoder (CokeBert-style 2-layer DK encoder).

Math per batch element b (see reference):
  q0 = q[b,0]                                    [768]
  qi_l = tanh(Wq_l @ q0 + bq_l)                  [100]   l in {2,1}
  w_l  = (Wk_l.T @ qi_l) / 10                    [100]   (Wk folded into a matvec)
  L0:  s2[e,n1,n2] = k2[e,n1,n2,:] . w2
       attn2 = masked leaky-softmax over n2
       c2[e,n1,:] = sum_n2 attn2 * v2[e,n1,n2,:]          [E,16,100]
  L1:  s1 = k1 . w1; attn1 likewise over n1
       c1[e,:] = sum_n1 attn1 * [v1|c2][e,n1,:]           [E,200]
  scatter: out[s] = c1[rank[s]] if input_ent[s]!=0 else 0

Sharding: data-parallel over B=8, one batch element per NeuronCore.

Design: everything heavy runs on TensorE in bf16; DVE only does softmax and
small glue. Host-side (free) layout prep makes that possible:
  * k2 is transposed+permuted on host to k2tp[c, T, n2, m] so each score
    matmul is lhsT=k2-chunk [100,128] (FWL bf16 weight load), rhs=w2 [100,1],
    landing scores directly as [128 G-partitions, 16 n2-cols] in PSUM
    (G = e*16+n1), the exact layout the grouped softmax wants.
  * v2 is row-chunk permuted on host to v2p[p, u, c] (row = 128u+p) so the
    attn2-weighted n2-reduction is a matmul per 128-row chunk:
    lhsT = block-diagonal attn2 [128, 8], rhs = v2 chunk [128, 100],
    accumulating c2 for 8 G-groups at a time.  The block-diagonal attn2 is
    built on TensorE too: transpose attn2 [128,16]->[16,128], broadcast rows
    mod 16 with a fixed shift matrix, then one DVE mask-multiply.
  * Layer 1 (k1/v1 + c2) uses the same machinery at 1/16 scale, and the
    final token scatter is the same permutation-matmul as the baseline.
All big DMAs are [*,>=2KB]-per-partition contiguous thanks to host packing.
"""
import numpy as np
import ml_dtypes

import concourse.bass as bass
import concourse.mybir as mybir
import concourse.tile as tile

F32 = mybir.dt.float32
BF16 = mybir.dt.bfloat16
I32 = mybir.dt.int32
AF = mybir.ActivationFunctionType
OP = mybir.AluOpType
AX = mybir.AxisListType

P = 128
D = 100            # K_V_DIM
NB = 16            # neighbors per group (N1 = N2 = 16)
S = 256            # sequence length
Q = 768            # query dim
E = 256            # entities
ET = E // P        # 2
NT2 = (E * NB) // P      # 32 k2 score tiles (G-chunks of 128)
NU2 = (E * NB * NB) // P  # 512 v2 row-chunks
NT1 = ET                 # 2 k1 score tiles
NU1 = (E * NB) // P      # 32 v1 row-chunks
VW = 32                  # v2 chunks per DMA tile
INV_SQRT_D = 0.1


def build_nc(repeat=1, phase="full", debug=False, spread=2):
    nc = bass.Bass()
    dma_engines = [nc.sync, nc.scalar, nc.gpsimd][:spread]

    # ---- I/O ----
    ent = nc.dram_tensor("ent", [1, S], I32, kind="ExternalInput")
    q0 = nc.dram_tensor("q0", [1, Q], F32, kind="ExternalInput")
    k2tp = nc.dram_tensor("k2tp", [D, NT2 * NB * P], BF16, kind="ExternalInput")
    v2p = nc.dram_tensor("v2p", [P, NU2 * D], BF16, kind="ExternalInput")
    k1tp = nc.dram_tensor("k1tp", [D, NT1 * NB * P], BF16, kind="ExternalInput")
    v1p = nc.dram_tensor("v1p", [P, NU1 * D], BF16, kind="ExternalInput")
    Wq2T = nc.dram_tensor("Wq2T", [Q, D], F32, kind="ExternalInput")
    bq2 = nc.dram_tensor("bq2", [D], F32, kind="ExternalInput")
    Wk2 = nc.dram_tensor("Wk2", [D, D], F32, kind="ExternalInput")
    Wq1T = nc.dram_tensor("Wq1T", [Q, D], F32, kind="ExternalInput")
    bq1 = nc.dram_tensor("bq1", [D], F32, kind="ExternalInput")
    Wk1 = nc.dram_tensor("Wk1", [D, D], F32, kind="ExternalInput")
    identb = nc.dram_tensor("identb", [P, P], BF16, kind="ExternalInput")
    shiftm = nc.dram_tensor("shiftm", [NB, P], BF16, kind="ExternalInput")
    maskbd = nc.dram_tensor("maskbd", [P, NB * 32], F32, kind="ExternalInput")
    iota_e = nc.dram_tensor("iota_e", [P, ET], F32, kind="ExternalInput")
    onesd = nc.dram_tensor("onesd", [1, P], F32, kind="ExternalInput")
    outp = nc.dram_tensor("outp", [S, 2 * D], F32, kind="ExternalOutput")
    if debug:
        dbg = {
            "dbg_S2": nc.dram_tensor("dbg_S2", [P, NT2 * NB], F32, kind="ExternalOutput"),
            "dbg_A2": nc.dram_tensor("dbg_A2", [P, NT2 * NB], F32, kind="ExternalOutput"),
            "dbg_S1": nc.dram_tensor("dbg_S1", [P, NT1 * NB], F32, kind="ExternalOutput"),
            "dbg_bd1": nc.dram_tensor("dbg_bd1", [P, NT1 * NB * 32], F32, kind="ExternalOutput"),
            "dbg_bd2": nc.dram_tensor("dbg_bd2", [P, NB * 32], F32, kind="ExternalOutput"),
            "dbg_c2": nc.dram_tensor("dbg_c2", [P, NU1 * D], F32, kind="ExternalOutput"),
            "dbg_c1": nc.dram_tensor("dbg_c1", [P, ET * 2 * D], F32, kind="ExternalOutput"),
            "dbg_PT": nc.dram_tensor("dbg_PT", [P, ET * S], F32, kind="ExternalOutput"),
            "dbg_w2": nc.dram_tensor("dbg_w2", [D, 1], F32, kind="ExternalOutput"),
        }

    with tile.TileContext(nc) as tc:
        with tc.tile_pool(name="cpool", bufs=1) as cp, \
             tc.tile_pool(name="work", bufs=1) as wk, \
             tc.tile_pool(name="k2st", bufs=4) as k2st, \
             tc.tile_pool(name="v2st", bufs=4) as v2st, \
             tc.tile_pool(name="pm", bufs=1, space="PSUM") as pm:

            def softmax_block(Sm, Ab, ncols, tagp):
                """Masked leaky-softmax over 16-col groups; Ab = bf16 out."""
                ng = ncols // NB
                g3 = lambda t: t[:].rearrange("p (g j) -> p g j", j=NB)
                zq = wk.tile([P, ncols], F32, tag=f"{tagp}_zq")
                nc.vector.tensor_scalar(out=zq[:], in0=Sm[:], scalar1=0.0,
                                        scalar2=-10000.0, op0=OP.is_equal,
                                        op1=OP.mult)
                sm = wk.tile([P, ncols], F32, tag=f"{tagp}_sm")
                nc.vector.tensor_tensor(out=sm[:], in0=Sm[:], in1=zq[:], op=OP.add)
                lt = wk.tile([P, ncols], F32, tag=f"{tagp}_lt")
                nc.vector.tensor_scalar(out=lt[:], in0=sm[:], scalar1=0.01,
                                        scalar2=None, op0=OP.mult)
                lr = wk.tile([P, ncols], F32, tag=f"{tagp}_lr")
                nc.vector.tensor_tensor(out=lr[:], in0=sm[:], in1=lt[:], op=OP.max)
                nm = wk.tile([P, ng], F32, tag=f"{tagp}_nm")
                nc.vector.tensor_reduce(out=nm[:], in_=g3(lr), axis=AX.X,
                                        op=OP.max, negate=True)
                xs = wk.tile([P, ncols], F32, tag=f"{tagp}_xs")
                nc.vector.tensor_tensor(out=g3(xs), in0=g3(lr),
                                        in1=nm[:].unsqueeze(2).to_broadcast([P, ng, NB]),
                                        op=OP.add)
                ex = wk.tile([P, ncols], F32, tag=f"{tagp}_ex")
                nc.scalar.activation(out=ex[:], in_=xs[:], func=AF.Exp,
                                     bias=0.0, scale=1.0)
                zz = wk.tile([P, ng], F32, tag=f"{tagp}_zz")
                nc.vector.reduce_sum(out=zz[:], in_=g3(ex), axis=AX.X)
                rz = wk.tile([P, ng], F32, tag=f"{tagp}_rz")
                nc.vector.reciprocal(out=rz[:], in_=zz[:])
                at = wk.tile([P, ncols], F32, tag=f"{tagp}_at")
                nc.vector.tensor_tensor(out=g3(at), in0=g3(ex),
                                        in1=rz[:].unsqueeze(2).to_broadcast([P, ng, NB]),
                                        op=OP.mult)
                mq = wk.tile([P, ncols], F32, tag=f"{tagp}_mq")
                nc.vector.tensor_scalar(out=mq[:], in0=at[:], scalar1=1.0 / NB,
                                        scalar2=None, op0=OP.not_equal)
                nc.vector.tensor_tensor(out=Ab[:], in0=at[:], in1=mq[:], op=OP.mult)

            def emit():
                dma_only = phase == "dma"
                dma_rr = [0]

                def big_dma(out, in_):
                    eng = dma_engines[dma_rr[0] % len(dma_engines)]
                    dma_rr[0] += 1
                    eng.dma_start(out=out, in_=in_)
                # ---------- constants ----------
                identc = cp.tile([P, P], BF16, tag="identc")
                nc.sync.dma_start(out=identc[:], in_=identb[:])
                shv = cp.tile([NB, P], BF16, tag="shv")
                nc.sync.dma_start(out=shv[:], in_=shiftm[:])
                mkb = cp.tile([P, NB * 32], F32, tag="mkb")
                nc.sync.dma_start(out=mkb[:], in_=maskbd[:])
                iotc = cp.tile([P, ET], F32, tag="iotc")
                nc.sync.dma_start(out=iotc[:], in_=iota_e[:])
                onesc = cp.tile([1, P], F32, tag="onesc")
                nc.sync.dma_start(out=onesc[:], in_=onesd[:])
                q0c = cp.tile([P, 6], F32, tag="q0c")
                nc.sync.dma_start(out=q0c[:], in_=q0[:].rearrange("a (j p) -> (a p) j", p=P))
                ent_i = cp.tile([1, S], I32, tag="ent_i")
                nc.sync.dma_start(out=ent_i[:], in_=ent[:])

                # ---------- q_i / w columns for both layers ----------
                wcols = {}
                for lname, WqT_d, bq_d, Wk_d in (("2", Wq2T, bq2, Wk2),
                                                 ("1", Wq1T, bq1, Wk1)):
                    wqt = cp.tile([P, 6, D], F32, tag=f"wqt{lname}")
                    nc.scalar.dma_start(out=wqt[:], in_=WqT_d[:].rearrange("(j p) m -> p j m", p=P))
                    bqc = cp.tile([D, 1], F32, tag=f"bqc{lname}")
                    nc.scalar.dma_start(out=bqc[:], in_=bq_d[:].unsqueeze(1))
                    wkt = cp.tile([D, D], F32, tag=f"wkt{lname}")
                    nc.scalar.dma_start(out=wkt[:], in_=Wk_d[:])
                    if dma_only:
                        continue
                    qi_ps = pm.tile([D, 1], F32, tag="misc")
                    for j in range(6):
                        nc.tensor.matmul(out=qi_ps[:], lhsT=wqt[:, j, :],
                                         rhs=q0c[:, j:j + 1],
                                         start=(j == 0), stop=(j == 5))
                    qi = cp.tile([D, 1], F32, tag=f"qi{lname}")
                    nc.scalar.activation(out=qi[:], in_=qi_ps[:], func=AF.Tanh,
                                         bias=bqc[:], scale=1.0)
                    w_ps = pm.tile([D, 1], F32, tag="misc")
                    nc.tensor.matmul(out=w_ps[:], lhsT=wkt[:], rhs=qi[:],
                                     start=True, stop=True)
                    wcol = cp.tile([D, 1], BF16, tag=f"wcol{lname}")
                    nc.scalar.activation(out=wcol[:], in_=w_ps[:], func=AF.Copy,
                                         scale=INV_SQRT_D)
                    wcols[lname] = wcol

                # ---------- scatter indices ----------
                if not dma_only:
                    ent_f = cp.tile([1, S], F32, tag="ent_f")
                    nc.vector.tensor_copy(out=ent_f[:], in_=ent_i[:])
                    msk = cp.tile([1, S], F32, tag="msk")
                    nc.vector.tensor_scalar(out=msk[:], in0=ent_f[:], scalar1=0.0,
                                            scalar2=None, op0=OP.not_equal)
                    csum = cp.tile([1, S], F32, tag="csum")
                    nc.vector.tensor_tensor_scan(out=csum[:], data0=msk[:],
                                                 data1=msk[:], initial=0.0,
                                                 op0=OP.add, op1=OP.bypass)
                    rank = cp.tile([1, S], F32, tag="rank")
                    nc.vector.tensor_tensor(out=rank[:], in0=csum[:], in1=msk[:],
                                            op=OP.mult)
                    nc.vector.tensor_scalar(out=rank[:], in0=rank[:], scalar1=-1.0,
                                            scalar2=float(E - 1), op0=OP.add,
                                            op1=OP.min)
                    rank_ps = pm.tile([P, S], F32, tag="misc")
                    nc.tensor.matmul(out=rank_ps[:], lhsT=onesc[:], rhs=rank[:],
                                     start=True, stop=True)
                    PT = cp.tile([P, ET, S], F32, tag="PT")
                    for kk in range(ET):
                        nc.vector.tensor_scalar(out=PT[:, kk, :], in0=rank_ps[:],
                                                scalar1=iotc[:, kk:kk + 1],
                                                scalar2=None, op0=OP.is_equal)

                # ---------- score matmuls (layer1 then layer0) ----------
                k1sb = cp.tile([D, NT1, NB, P], BF16, tag="k1sb")
                big_dma(k1sb[:], k1tp[:])
                v1sb = cp.tile([P, NU1, D], BF16, tag="v1sb")
                big_dma(v1sb[:], v1p[:])

                S1 = cp.tile([P, NT1 * NB], F32, tag="S1")
                S2 = cp.tile([P, NT2 * NB], F32, tag="S2")
                with tc.tile_pool(name="ps", bufs=1, space="PSUM") as ps:
                    if not dma_only:
                        s1ps = ps.tile([P, NT1 * NB], F32, tag="s1ps")
                        for T in range(NT1):
                            for n1 in range(NB):
                                nc.tensor.matmul(
                                    out=s1ps[:, NB * T + n1:NB * T + n1 + 1],
                                    lhsT=k1sb[:, T, n1, :], rhs=wcols["1"][:],
                                    start=True, stop=True)
                        nc.scalar.activation(out=S1[:], in_=s1ps[:], func=AF.Copy)

                    for T in range(NT2):
                        kt = k2st.tile([D, NB, P], BF16, tag="kt")
                        big_dma(kt[:], k2tp[:, NB * P * T:NB * P * (T + 1)])
                        if dma_only:
                            continue
                        if T % 8 == 0:
                            s2ps = ps.tile([P, P], F32, tag="s2ps", bufs=2)
                        for n2 in range(NB):
                            nc.tensor.matmul(
                                out=s2ps[:, NB * (T % 8) + n2:NB * (T % 8) + n2 + 1],
                                lhsT=kt[:, n2, :], rhs=wcols["2"][:],
                                start=True, stop=True)
                        if T % 8 == 7:
                            nc.scalar.activation(out=S2[:, NB * (T - 7):NB * (T + 1)],
                                                 in_=s2ps[:], func=AF.Copy)

                # ---------- softmaxes ----------
                A1b = cp.tile([P, NT1 * NB], BF16, tag="A1b")
                A2b = cp.tile([P, NT2 * NB], BF16, tag="A2b")
                if phase == "scores":
                    # all remaining DMAs + drain scores; no further compute
                    for w in range(NU2 // VW):
                        vt = v2st.tile([P, VW, D], BF16, tag="vt")
                        big_dma(vt[:], v2p[:, VW * D * w:VW * D * (w + 1)])
                    nc.sync.dma_start(out=outp[0:P, 0:NT1 * NB], in_=S1[:])
                    nc.scalar.dma_start(out=outp[0:P, 0:2 * D],
                                        in_=S2[:, 0:2 * D])
                    return
                if not dma_only:
                    softmax_block(S1, A1b, NT1 * NB, "x1")
                    softmax_block(S2, A2b, NT2 * NB, "x2")
                if phase == "sm":
                    for w in range(NU2 // VW):
                        vt = v2st.tile([P, VW, D], BF16, tag="vt")
                        big_dma(vt[:], v2p[:, VW * D * w:VW * D * (w + 1)])
                    a2f = wk.tile([P, 2 * D], F32, tag="a2f_dr")
                    nc.vector.tensor_copy(out=a2f[:], in_=A2b[:, 0:2 * D])
                    nc.sync.dma_start(out=outp[0:P, :], in_=a2f[:])
                    return

                # ---------- main pipeline ----------
                # Banded block-diagonal attn tiles: column layout (j, mm)
                # with j = chunk-in-tile (16), mm = out-row-in-32-block (32);
                # chunk j's lhsT is the contiguous [128, 32] slice at 32*j,
                # nonzero only in band mm in [8*(j%4), +8).  Built by one
                # broadcast matmul (shift16 x replicated-attn view) + one
                # masked multiply.
                c2sb = cp.tile([P, NU1, D], BF16, tag="c2sb")
                c1sb = cp.tile([P, ET, 2 * D], F32, tag="c1sb")
                BW = NB * 32    # banded tile width (512)

                def bcast_view(at):
                    # a2t [16, 128] -> [16, (jj 4, qq 4, mm 32)] with qq
                    # a stride-0 repeat: col (j=4jj+qq, mm) reads 32*jj+mm.
                    v = at[:].rearrange("n (jj mm) -> n jj mm", mm=32)
                    return v.unsqueeze(2).to_broadcast([NB, 4, 4, 32])

                with tc.tile_pool(name="tb", bufs=2, space="PSUM") as tb, \
                     tc.tile_pool(name="pb", bufs=2, space="PSUM") as pb, \
                     tc.tile_pool(name="pc", bufs=2, space="PSUM") as pc, \
                     tc.tile_pool(name="pd", bufs=1, space="PSUM") as pd:

                    # layer-1 banded attn tiles
                    bd1 = cp.tile([P, NT1, BW], BF16, tag="bd1")
                    if not dma_only:
                        for t1 in range(NT1):
                            tp1 = tb.tile([NB, P], F32, tag="tps")
                            nc.tensor.matmul(out=tp1[:], lhsT=A1b[:, NB * t1:NB * (t1 + 1)],
                                             rhs=identc[:], start=True, stop=True)
                            a1t = wk.tile([NB, P], BF16, tag="a1t", bufs=2)
                            nc.scalar.activation(out=a1t[:], in_=tp1[:], func=AF.Copy)
                            bc1 = pb.tile([P, BW], F32, tag="bc2")
                            nc.tensor.matmul(out=bc1[:], lhsT=shv[:], rhs=bcast_view(a1t),
                                             start=True, stop=True)
                            nc.vector.tensor_tensor(out=bd1[:, t1, :], in0=bc1[:],
                                                    in1=mkb[:], op=OP.mult)

                    # v2 stream: bd2(t') built one stage ahead of the chunk
                    # matmuls; layer-1 c1 matmuls trail two stages.
                    bds = {}
                    c1p = None

                    def stage_bd(t):
                        vw = NT2 // (NU2 // VW)  # t' per v2 dma tile (=2)
                        if t % vw == 0:
                            vt = v2st.tile([P, VW, D], BF16, tag="vt")
                            big_dma(vt[:],
                                    v2p[:, VW * D * (t // vw):VW * D * (t // vw + 1)])
                            stage_bd.vt = vt
                        if dma_only:
                            return
                        tps = tb.tile([NB, P], F32, tag="tps")
                        nc.tensor.matmul(out=tps[:], lhsT=A2b[:, NB * t:NB * (t + 1)],
                                         rhs=identc[:], start=True, stop=True)
                        a2t = wk.tile([NB, P], BF16, tag="a2t", bufs=2)
                        nc.scalar.activation(out=a2t[:], in_=tps[:], func=AF.Copy)
                        bc = pb.tile([P, BW], F32, tag="bc2")
                        nc.tensor.matmul(out=bc[:], lhsT=shv[:], rhs=bcast_view(a2t),
                                         start=True, stop=True)
                        bd = wk.tile([P, BW], BF16, tag="bd", bufs=2)
                        nc.vector.tensor_tensor(out=bd[:], in0=bc[:], in1=mkb[:],
                                                op=OP.mult)
                        if debug and t == 0:
                            dt_ = cp.tile([P, BW], F32, tag="dbg_bd2t")
                            nc.vector.tensor_copy(out=dt_[:], in_=bd[:])
                            nc.sync.dma_start(out=dbg["dbg_bd2"][:], in_=dt_[:])
                        bds[t] = (bd, stage_bd.vt)

                    def stage_v2(t):
                        # PSUM out blocks may only start at partition 0/32/64,
                        # so the 128 G-rows of a tile live as two 64-row
                        # halves side by side in the free dim.
                        if dma_only:
                            return
                        bd, vt = bds.pop(t)
                        cps = pc.tile([64, 2, D], F32, tag="c2ps")
                        for j in range(NB):
                            u = NB * t + j
                            q = j // 4
                            blk = slice(32 * (q % 2), 32 * (q % 2) + 32)
                            nc.tensor.matmul(out=cps[blk, q // 2, :],
                                             lhsT=bd[:, 32 * j:32 * (j + 1)],
                                             rhs=vt[:, u % VW, :],
                                             start=(j % 4 == 0), stop=(j % 4 == 3))
                        nc.scalar.activation(out=c2sb[0:64, t, :], in_=cps[:, 0, :],
                                             func=AF.Copy)
                        nc.scalar.activation(out=c2sb[64:P, t, :], in_=cps[:, 1, :],
                                             func=AF.Copy)

                    def stage_c1_quad(qt):
                        # One PSUM zero-region only supports sequential
                        # start->stop accumulation groups, so emit a quad's
                        # 4 v1-half matmuls, then its 4 c2-half matmuls.
                        if dma_only:
                            return
                        nonlocal c1p
                        t1 = qt // 4
                        q = qt % 4
                        if q == 0:
                            c1p = pd.tile([64, 2, 2 * D], F32, tag="c1ps")
                        blk = slice(32 * (q % 2), 32 * (q % 2) + 32)
                        for half, rhs_of in ((0, lambda t: v1sb[:, t, :]),
                                             (1, lambda t: c2sb[:, t, :])):
                            for qq in range(4):
                                j1 = 4 * q + qq
                                t = NB * t1 + j1
                                nc.tensor.matmul(
                                    out=c1p[blk, q // 2, D * half:D * (half + 1)],
                                    lhsT=bd1[:, t1, 32 * j1:32 * (j1 + 1)],
                                    rhs=rhs_of(t), start=(qq == 0), stop=(qq == 3))
                        if q == 3:
                            nc.vector.tensor_copy(out=c1sb[0:64, t1, :],
                                                  in_=c1p[:, 0, :])
                            nc.vector.tensor_copy(out=c1sb[64:P, t1, :],
                                                  in_=c1p[:, 1, :])

                    do_v2 = phase in ("v2", "c1", "full")
                    do_c1 = phase in ("c1", "full")
                    for t in range(NT2):
                        stage_bd(t)
                        if t >= 1 and do_v2:
                            stage_v2(t - 1)
                            if (t - 1) % 4 == 3 and do_c1:
                                stage_c1_quad((t - 1) // 4)
                    if do_v2:
                        stage_v2(NT2 - 1)
                        if do_c1:
                            stage_c1_quad((NT2 - 1) // 4)
                    if phase == "bd":
                        bdl, _ = bds[NT2 - 1]
                        bdf = wk.tile([P, 2 * D], F32, tag="bd_dr")
                        nc.vector.tensor_copy(out=bdf[:], in_=bdl[:, 0:2 * D])
                        nc.sync.dma_start(out=outp[0:P, :], in_=bdf[:])
                        return
                    if phase == "v2":
                        c2f_ = wk.tile([P, 2 * D], F32, tag="c2_dr")
                        nc.vector.tensor_copy(
                            out=c2f_[:],
                            in_=c2sb[:, NU1 - 2:NU1, :].rearrange("p a b -> p (a b)"))
                        nc.sync.dma_start(out=outp[0:P, :], in_=c2f_[:])
                        return
                    if phase == "c1":
                        nc.sync.dma_start(out=outp[0:P, :],
                                          in_=c1sb[:, 0, :])
                        return

                    if debug and not dma_only:
                        nc.sync.dma_start(out=dbg["dbg_S2"][:], in_=S2[:])
                        nc.sync.dma_start(out=dbg["dbg_S1"][:], in_=S1[:])
                        a2f = cp.tile([P, NT2 * NB], F32, tag="dbg_a2f")
                        nc.vector.tensor_copy(out=a2f[:], in_=A2b[:])
                        nc.sync.dma_start(out=dbg["dbg_A2"][:], in_=a2f[:])
                        bd1f = cp.tile([P, NT1 * BW], F32, tag="dbg_bd1f")
                        nc.vector.tensor_copy(out=bd1f[:], in_=bd1[:].rearrange("p a b -> p (a b)"))
                        nc.sync.dma_start(out=dbg["dbg_bd1"][:], in_=bd1f[:])
                        c2f = cp.tile([P, NU1 * D], F32, tag="dbg_c2f")
                        nc.vector.tensor_copy(out=c2f[:], in_=c2sb[:].rearrange("p a b -> p (a b)"))
                        nc.sync.dma_start(out=dbg["dbg_c2"][:], in_=c2f[:])
                        nc.sync.dma_start(out=dbg["dbg_c1"][:],
                                          in_=c1sb[:].rearrange("p a b -> p (a b)"))
                        nc.sync.dma_start(out=dbg["dbg_PT"][:],
                                          in_=PT[:].rearrange("p a b -> p (a b)"))
                        w2f = cp.tile([D, 1], F32, tag="dbg_w2f")
                        nc.vector.tensor_copy(out=w2f[:], in_=wcols["2"][:])
                        nc.sync.dma_start(out=dbg["dbg_w2"][:], in_=w2f[:])

                    # ---------- scatter ----------
                    for hh in range(S // P):
                        ops = pm.tile([P, 2 * D], F32, tag="misc")
                        if not dma_only:
                            for kk in range(ET):
                                nc.tensor.matmul(out=ops[:],
                                                 lhsT=PT[:, kk, P * hh:P * (hh + 1)],
                                                 rhs=c1sb[:, kk, :],
                                                 start=(kk == 0), stop=(kk == ET - 1))
                            osb = wk.tile([P, 2 * D], F32, tag="osb", bufs=2)
                            nc.scalar.activation(out=osb[:], in_=ops[:], func=AF.Copy)
                            deng = nc.sync if hh % 2 == 0 else nc.scalar
                            deng.dma_start(out=outp[P * hh:P * (hh + 1), :],
                                           in_=osb[:])

            for _rep in range(repeat):
                emit()

    _split_multi_waits(nc)
    return nc


def _split_multi_waits(nc):
    """This walrus build allows at most ONE sync-wait command per
    instruction; hoist extras onto standalone EventSemaphore waits."""
    n = 0
    for bb in nc.m.functions[0].blocks:
        insts = bb.instructions
        i = 0
        while i < len(insts):
            ins = insts[i]
            si = ins.sync_info
            if si is not None and si.on_wait and len(si.on_wait) >= 2:
                extras, keep = list(si.on_wait[:-1]), [si.on_wait[-1]]
                for w in extras:
                    e = mybir.InstEventSemaphore(
                        name=nc.get_next_instruction_name(), ins=[], outs=[])
                    e.engine = ins.engine
                    e.sync_info = mybir.SyncInfo(on_wait=[w], on_update=[])
                    insts.insert(i, e)
                    i += 1
                    n += 1
                ins.sync_info = mybir.SyncInfo(on_wait=keep,
                                               on_update=list(si.on_update))
            i += 1
    return n


# ------------------------------------------------------------------
# host-side wrapper
# ------------------------------------------------------------------
_NC_CACHE = {}


def _get_nc(repeat=1, phase="full"):
    key = (repeat, phase)
    if key not in _NC_CACHE:
        _NC_CACHE[key] = build_nc(repeat=repeat, phase=phase)
    return _NC_CACHE[key]


def _constants():
    ident = np.eye(P, dtype=ml_dtypes.bfloat16)
    shift = np.zeros((NB, P), dtype=ml_dtypes.bfloat16)
    for k in range(NB):
        shift[k, np.arange(P) % NB == k] = 1.0
    # banded mask [128, 512]: col = 32*j + mm; 1 iff p//16 == mm - 8*(j%4)
    maskbd = np.zeros((P, NB, 32), dtype=np.float32)
    for p in range(P):
        for j in range(NB):
            mm = 8 * (j % 4) + p // NB
            maskbd[p, j, mm] = 1.0
    maskbd = maskbd.reshape(P, NB * 32)
    iot = np.zeros((P, ET), np.float32)
    for kk in range(ET):
        iot[:, kk] = np.arange(P) + P * kk
    ones = np.ones((1, P), np.float32)
    return ident, shift, maskbd, iot, ones


def _in_map_for_core(b, inputs):
    bf = ml_dtypes.bfloat16
    ident, shift, maskbd, iot, ones = _constants()
    k2 = np.asarray(inputs["k2"][b], np.float32).reshape(NT2, P, NB, D)
    k2tp = np.ascontiguousarray(k2.transpose(3, 0, 2, 1)).astype(bf)
    v2 = np.asarray(inputs["v2"][b], np.float32).reshape(NU2, P, D)
    v2p = np.ascontiguousarray(v2.transpose(1, 0, 2)).astype(bf)
    k1 = np.asarray(inputs["k1"][b], np.float32).reshape(NT1, P, NB, D)
    k1tp = np.ascontiguousarray(k1.transpose(3, 0, 2, 1)).astype(bf)
    v1 = np.asarray(inputs["v1"][b], np.float32).reshape(NU1, P, D)
    v1p = np.ascontiguousarray(v1.transpose(1, 0, 2)).astype(bf)
    return {
        "ent": np.asarray(inputs["input_ent"][b]).astype(np.int32).reshape(1, S),
        "q0": np.ascontiguousarray(np.asarray(inputs["q"][b, 0], np.float32)).reshape(1, Q),
        "k2tp": k2tp.reshape(D, NT2 * NB * P),
        "v2p": v2p.reshape(P, NU2 * D),
        "k1tp": k1tp.reshape(D, NT1 * NB * P),
        "v1p": v1p.reshape(P, NU1 * D),
        "Wq2T": np.ascontiguousarray(np.asarray(inputs["Wq2"], np.float32).T),
        "bq2": np.ascontiguousarray(np.asarray(inputs["bq2"], np.float32)),
        "Wk2": np.ascontiguousarray(np.asarray(inputs["Wk2"], np.float32)),
        "Wq1T": np.ascontiguousarray(np.asarray(inputs["Wq1"], np.float32).T),
        "bq1": np.ascontiguousarray(np.asarray(inputs["bq1"], np.float32)),
        "Wk1": np.ascontiguousarray(np.asarray(inputs["Wk1"], np.float32)),
        "identb": ident,
        "shiftm": shift,
        "maskbd": maskbd,
        "iota_e": iot,
        "onesd": ones,
    }


def kernel(input_ent, q, k1, v1, k2, v2, Wq2, bq2, Wk2, Wq1, bq1, Wk1, **kw):
    from concourse.bass_utils import run_bass_kernel_spmd

    inputs = dict(input_ent=np.asarray(input_ent), q=np.asarray(q, np.float32),
                  k1=np.asarray(k1, np.float32), v1=np.asarray(v1, np.float32),
                  k2=np.asarray(k2, np.float32), v2=np.asarray(v2, np.float32),
                  Wq2=np.asarray(Wq2, np.float32), bq2=np.asarray(bq2, np.float32),
                  Wk2=np.asarray(Wk2, np.float32), Wq1=np.asarray(Wq1, np.float32),
                  bq1=np.asarray(bq1, np.float32), Wk1=np.asarray(Wk1, np.float32))
    B = inputs["input_ent"].shape[0]
    nc = _get_nc(repeat=1, phase=kw.get("phase", "full"))
    in_maps = [_in_map_for_core(b, inputs) for b in range(B)]
    res = run_bass_kernel_spmd(nc, in_maps, core_ids=list(range(B)),
                               trace=kw.get("trace", False))
    out = np.stack([res.results[b]["outp"] for b in range(B)], axis=0)
    if kw.get("return_res", False):
        return out, res
    return out

